# revision 20
# baseline (speedup 1.0000x reference)
"""Bass/Trainium2 kernel for nn_BranchedPolicyNetwork.

Computes out = tanh(features @ Wr + br) where
  features: [32768, 1024] f32
  W:        [64, 2, 1024] f32  (stacked per-branch Linear(L, 2) weights)
  b:        [64, 2] f32
returning (out[..., 0], out[..., 1]) as two [32768, 64] f32 arrays.

Strategy: data-parallel over batch across 8 NeuronCores (4096 rows each).
The TensorEngine contracts over the partition dim, so features are repacked
host-side into a transposed, tile-contiguous layout (free w.r.t. HW time).

Precision: x in fp8 e3m4 (4 mantissa bits), w in fp16, fp32 PSUM
accumulation, output stored as fp16.  Measured end-to-end rel_l2 vs the
fp32 reference on the actual inputs: 1.27e-2 (gate is 2e-2; inputs are a
fixed seed so this is deterministic).  e3m4 covers x ~ N(0,1) fine
(absmax 5.4 < 15.5 max normal) and halves HBM x traffic vs fp16:
4.19 MB x + 0.26 MB w in, 1 MB out per core.  Mixed fp8xfp16 matmuls run
at the standard 1 row/cycle PE rate, so PE work is unchanged (~13.8 us at
full clock) and the kernel stays stream-bound.

The kernel is otherwise the measured-best fp16 structure: single Sync-ring
x stream (two HWDGE queues trigger harder power throttling and regress),
ko-major matmuls into one [CH,1024] PSUM tile per chunk (215 ns cadence;
split PSUM pool tiles measured 258 ns), per-chunk tanh + fp16 store on the
Scalar ring, and ~10 warmup matmuls to ramp the PE clock while the first
loads stream in (removing them collapses the DMA stream: the half-clock PE
runs ~100% duty and the power manager clamps DMA to ~50%).
"""

import sys

for _p in ("/opt/trn_rl_repo", "/root/.axon_site"):
    if _p not in sys.path:
        sys.path.insert(0, _p)

import ml_dtypes
import numpy as np

import concourse.mybir as mybir
import concourse.tile as tile
from concourse import bacc
from concourse.bass_utils import run_bass_kernel_spmd

# Problem shapes (hardcoded per contract)
B, L, A = 32768, 1024, 64
NCORES = 8
BS = B // NCORES          # 4096 batch rows per core
KO = L // 128             # 8 contraction slices
CH = 2 * A                # 128 output channels (c = k*64 + a)

F32 = mybir.dt.float32
F16 = mybir.dt.float16
F8 = mybir.dt.float8e3   # e3m4 <-> ml_dtypes.float8_e3m4
NP_F8 = ml_dtypes.float8_e3m4

CHUNKS = [1024, 1024, 1024, 512, 512]
assert sum(CHUNKS) == BS
CN_MAX = max(CHUNKS)
MM_N = 512  # moving free dim per matmul (one fp32 PSUM bank)

_NC = None


def _build_nc():
    nc = bacc.Bacc()
    # x is packed chunk-major on the host: for each chunk (cn columns), the
    # per-partition bytes are one contiguous (ko, n) block of KO*cn elements.
    x8 = nc.dram_tensor("x8", [128, KO * BS], F8, kind="ExternalInput")
    wh = nc.dram_tensor("wh", [128, KO, CH], F16, kind="ExternalInput")
    bvec = nc.dram_tensor("bias", [CH, 1], F32, kind="ExternalInput")
    out = nc.dram_tensor("out", [CH, BS], F16, kind="ExternalOutput")

    with tile.TileContext(nc) as tc:
        with (
            tc.tile_pool(name="consts", bufs=1) as consts,
            tc.tile_pool(name="xp", bufs=5) as xp,
            tc.tile_pool(name="op", bufs=3) as op,
            tc.tile_pool(name="ps", bufs=3, space="PSUM") as ps,
            tc.tile_pool(name="warm", bufs=1, space="PSUM") as warm_ps,
        ):
            # PE warmup: ~10 dependency-free matmuls on zeroed tiles while
            # the first loads stream in (see module docstring).
            w_warm = consts.tile([128, CH], F16)
            nc.vector.memset(w_warm[:], 0.0)
            x_warm = consts.tile([128, MM_N], F16)
            nc.gpsimd.memset(x_warm[:], 0.0)
            pw = warm_ps.tile([CH, MM_N], F32)
            # 6 warmups bridge the PE from the preamble barrier (~7.8us) to
            # the first real matmul (~9.5us) with no idle gap: the HAM clock
            # gate un-throttles ~3.4us after sustained PE activity begins, so
            # any gap both wastes the warmup and delays full clock.
            for i in range(6):
                nc.tensor.matmul(
                    pw[:], w_warm[:], x_warm[:], start=(i == 0), stop=(i == 5)
                )
            # Constants on the Scalar ring.  w is split so the 32 KB ko=0
            # slice (all the first matmuls need) lands ~1.5us before the
            # full 256 KB load would.
            wh_sb = consts.tile([128, KO, CH], F16)
            nc.scalar.dma_start(wh_sb[:, 0:1], wh[:, 0:1])
            b_sb = consts.tile([CH, 1], F32)

            # Issue ALL x loads up front on the Sync ring: with bufs=4 and 4
            # chunks, every x tile has its own SBUF slot, so the ring streams
            # continuously at HBM rate.  Sub-DMA pieces keep dependency
            # granularity fine at the head; hs=4 gives 4 KB per-partition
            # lines (the fastest measured packet size) for the bulk.
            x_tiles = []
            n0 = 0
            for ci, cn in enumerate(CHUNKS):
                off = KO * n0
                src = x8[:, off : off + KO * cn].rearrange(
                    "p (ko n) -> p ko n", ko=KO
                )
                x_sb = xp.tile([128, KO, CN_MAX], F8, tag="x8", name="x_sb")[:, :, :cn]
                # 4 KB per-partition lines for the bulk; finer pieces at the
                # head of chunk 0 so the PE's first real matmul starts as
                # early as possible.  For the 512-col tail chunks hs=8 is one
                # descriptor with 4 KB lines.
                if ci == 0:
                    # Head piece rides the (otherwise idle) Scalar ring so the
                    # PE's first real matmuls don't wait on the Sync ring's
                    # power-ramped delivery; sustained dual-queue streaming
                    # throttles, a 256 KB head overlap does not.
                    nc.scalar.dma_start(x_sb[:, 0:2], src[:, 0:2])
                    nc.scalar.dma_start(wh_sb[:, 1:], wh[:, 1:])
                    nc.scalar.dma_start(b_sb[:], bvec[:])
                    splits = [(2, 4), (4, 6), (6, 8)]
                else:
                    hs = 4 if cn == 1024 else 8
                    splits = [(k0, k0 + hs) for k0 in range(0, KO, hs)]
                for k0, k1 in splits:
                    nc.sync.dma_start(
                        x_sb[:, k0:k1], src[:, k0:k1]
                    )
                x_tiles.append(x_sb)
                n0 += cn

            n0 = 0
            for ci, cn in enumerate(CHUNKS):
                x_sb = x_tiles[ci]
                pt = ps.tile([CH, CN_MAX], F32, tag="pt", name="pt")[:, :cn]
                for ko in range(KO):
                    for s0 in range(0, cn, MM_N):
                        s1 = min(s0 + MM_N, cn)
                        # start/stop are per PSUM slab (bank region)
                        nc.tensor.matmul(
                            pt[:, s0:s1],
                            wh_sb[:, ko],
                            x_sb[:, ko, s0:s1],
                            start=(ko == 0),
                            stop=(ko == KO - 1),
                        )
                o_sb = op.tile([CH, CN_MAX], F16, tag="o", name="o_sb")[:, :cn]
                nc.scalar.activation(
                    o_sb[:],
                    pt[:],
                    mybir.ActivationFunctionType.Tanh,
                    bias=b_sb[:, 0:1],
                    scale=1.0,
                )
                # Store via the ACT engine's HWDGE ring: the store depends on
                # the activation anyway, and this keeps the Sync ring free to
                # stream x loads.
                nc.scalar.dma_start(out[:, n0 : n0 + cn], o_sb[:])
                n0 += cn
    nc.compile()
    return nc


def _get_nc():
    global _NC
    if _NC is None:
        _NC = _build_nc()
    return _NC


def _pack_x(shard8):
    # shard8 [BS, L] -> chunk-major [128, KO*BS]: per partition p, chunk c
    # occupies a contiguous (ko, n) block.
    shT = shard8.T  # [L, BS] view
    parts = []
    n0 = 0
    for cn in CHUNKS:
        blk = (
            shT[:, n0 : n0 + cn]
            .reshape(KO, 128, cn)
            .transpose(1, 0, 2)
            .reshape(128, KO * cn)
        )
        parts.append(blk)
        n0 += cn
    return np.ascontiguousarray(np.concatenate(parts, axis=1))


def _shard_inputs(features, W, b):
    features = np.ascontiguousarray(features, dtype=np.float32)
    W = np.ascontiguousarray(W, dtype=np.float32)
    b = np.ascontiguousarray(b, dtype=np.float32)

    # Wr[l, c] with c = k*A + a; fp16, device layout [p, ko, c]
    wr = W.transpose(2, 1, 0).reshape(L, CH)
    wr_h = wr.astype(np.float16)
    wh_dev = np.ascontiguousarray(wr_h.reshape(KO, 128, CH).transpose(1, 0, 2))
    b_dev = np.ascontiguousarray(b.transpose(1, 0).reshape(CH, 1))

    in_maps = []
    for i in range(NCORES):
        sh = features[i * BS : (i + 1) * BS]  # [BS, L]
        sh8 = sh.astype(NP_F8)
        in_maps.append(
            {
                "x8": _pack_x(sh8),
                "wh": wh_dev,
                "bias": b_dev,
            }
        )
    return in_maps


def _gather(results):
    out0 = np.empty((B, A), dtype=np.float32)
    out1 = np.empty((B, A), dtype=np.float32)
    for i, r in enumerate(results):
        arr = r["out"].T.astype(np.float32)  # [CH, BS] -> [BS, CH]
        out0[i * BS : (i + 1) * BS] = arr[:, :A]
        out1[i * BS : (i + 1) * BS] = arr[:, A:]
    return out0, out1


def _run(inputs, trace=False, trace_cores=None):
    nc = _get_nc()
    in_maps = _shard_inputs(inputs["features"], inputs["W"], inputs["b"])
    res = run_bass_kernel_spmd(
        nc,
        in_maps,
        core_ids=list(range(NCORES)),
        trace=trace,
        trace_cores=trace_cores,
    )
    return _gather(res.results), res


def kernel(features, W, b):
    (out0, out1), _ = _run({"features": features, "W": W, "b": b})
    return out0, out1


# revision 21
# speedup vs baseline: 1.0816x; 1.0816x over previous
"""Bass/Trainium2 kernel for nn_BranchedPolicyNetwork.

Computes out = tanh(features @ Wr + br) where
  features: [32768, 1024] f32
  W:        [64, 2, 1024] f32  (stacked per-branch Linear(L, 2) weights)
  b:        [64, 2] f32
returning (out[..., 0], out[..., 1]) as two [32768, 64] f32 arrays.

Strategy: data-parallel over batch across 8 NeuronCores (4096 rows each).
The TensorEngine contracts over the partition dim, so features are repacked
host-side into a transposed, tile-contiguous layout (free w.r.t. HW time).

Precision: x in fp8 e3m4 (4 mantissa bits), w in fp16, fp32 PSUM
accumulation, output stored as fp16.  Measured end-to-end rel_l2 vs the
fp32 reference on the actual inputs: 1.27e-2 (gate is 2e-2; inputs are a
fixed seed so this is deterministic).  e3m4 covers x ~ N(0,1) fine
(absmax 5.4 < 15.5 max normal) and halves HBM x traffic vs fp16:
4.19 MB x + 0.26 MB w in, 1 MB out per core.  Mixed fp8xfp16 matmuls run
at the standard 1 row/cycle PE rate, so PE work is unchanged (~13.8 us at
full clock) and the kernel stays stream-bound.

The kernel is otherwise the measured-best fp16 structure: single Sync-ring
x stream (two HWDGE queues trigger harder power throttling and regress),
ko-major matmuls into one [CH,1024] PSUM tile per chunk (215 ns cadence;
split PSUM pool tiles measured 258 ns), per-chunk tanh + fp16 store on the
Scalar ring, and ~10 warmup matmuls to ramp the PE clock while the first
loads stream in (removing them collapses the DMA stream: the half-clock PE
runs ~100% duty and the power manager clamps DMA to ~50%).
"""

import sys

for _p in ("/opt/trn_rl_repo", "/root/.axon_site"):
    if _p not in sys.path:
        sys.path.insert(0, _p)

import ml_dtypes
import numpy as np

import concourse.mybir as mybir
import concourse.tile as tile
from concourse import bacc
from concourse.bass_utils import run_bass_kernel_spmd

# Problem shapes (hardcoded per contract)
B, L, A = 32768, 1024, 64
NCORES = 8
BS = B // NCORES          # 4096 batch rows per core
KO = L // 128             # 8 contraction slices
CH = 2 * A                # 128 output channels (c = k*64 + a)

F32 = mybir.dt.float32
F16 = mybir.dt.float16
F8 = mybir.dt.float8e3   # e3m4 <-> ml_dtypes.float8_e3m4
NP_F8 = ml_dtypes.float8_e3m4

CHUNKS = [1024, 1024, 1024, 512, 512]
assert sum(CHUNKS) == BS
CN_MAX = max(CHUNKS)
MM_N = 512  # moving free dim per matmul (one fp32 PSUM bank)

_NC = None


def _build_nc():
    nc = bacc.Bacc()
    # x is packed chunk-major on the host: for each chunk (cn columns), the
    # per-partition bytes are one contiguous (ko, n) block of KO*cn elements.
    x8 = nc.dram_tensor("x8", [128, KO * BS], F8, kind="ExternalInput")
    wh = nc.dram_tensor("wh", [128, KO, CH], F16, kind="ExternalInput")
    bvec = nc.dram_tensor("bias", [CH, 1], F32, kind="ExternalInput")
    out = nc.dram_tensor("out", [CH, BS], F16, kind="ExternalOutput")

    with tile.TileContext(nc) as tc:
        with (
            tc.tile_pool(name="consts", bufs=1) as consts,
            tc.tile_pool(name="xp", bufs=5) as xp,
            tc.tile_pool(name="op", bufs=3) as op,
            tc.tile_pool(name="ps", bufs=3, space="PSUM") as ps,
            tc.tile_pool(name="warm", bufs=1, space="PSUM") as warm_ps,
        ):
            # PE warmup: ~10 dependency-free matmuls on zeroed tiles while
            # the first loads stream in (see module docstring).
            w_warm = consts.tile([128, CH], F16)
            nc.vector.memset(w_warm[:], 0.0)
            x_warm = consts.tile([128, MM_N], F16)
            nc.gpsimd.memset(x_warm[:], 0.0)
            pw = warm_ps.tile([CH, MM_N], F32)
            # 6 warmups bridge the PE from the preamble barrier (~7.8us) to
            # the first real matmul (~9.5us) with no idle gap: the HAM clock
            # gate un-throttles ~3.4us after sustained PE activity begins, so
            # any gap both wastes the warmup and delays full clock.
            for i in range(6):
                nc.tensor.matmul(
                    pw[:], w_warm[:], x_warm[:], start=(i == 0), stop=(i == 5)
                )
            # Constants on the Scalar ring.  w is split so the 32 KB ko=0
            # slice (all the first matmuls need) lands ~1.5us before the
            # full 256 KB load would.
            wh_sb = consts.tile([128, KO, CH], F16)
            nc.scalar.dma_start(wh_sb[:, 0:1], wh[:, 0:1])
            nc.scalar.dma_start(wh_sb[:, 1:], wh[:, 1:])
            b_sb = consts.tile([CH, 1], F32)
            nc.scalar.dma_start(b_sb[:], bvec[:])

            # Issue ALL x loads up front on the Sync ring: with bufs=4 and 4
            # chunks, every x tile has its own SBUF slot, so the ring streams
            # continuously at HBM rate.  Sub-DMA pieces keep dependency
            # granularity fine at the head; hs=4 gives 4 KB per-partition
            # lines (the fastest measured packet size) for the bulk.
            x_tiles = []
            n0 = 0
            for ci, cn in enumerate(CHUNKS):
                off = KO * n0
                src = x8[:, off : off + KO * cn].rearrange(
                    "p (ko n) -> p ko n", ko=KO
                )
                x_sb = xp.tile([128, KO, CN_MAX], F8, tag="x8", name="x_sb")[:, :, :cn]
                # 4 KB per-partition lines for the bulk; finer pieces at the
                # head of chunk 0 so the PE's first real matmul starts as
                # early as possible.  For the 512-col tail chunks hs=8 is one
                # descriptor with 4 KB lines.
                if ci == 0:
                    splits = [(0, 1), (1, 2), (2, 4), (4, 6), (6, 8)]
                else:
                    hs = 4 if cn == 1024 else 8
                    splits = [(k0, k0 + hs) for k0 in range(0, KO, hs)]
                for k0, k1 in splits:
                    nc.sync.dma_start(
                        x_sb[:, k0:k1], src[:, k0:k1]
                    )
                x_tiles.append(x_sb)
                n0 += cn

            n0 = 0
            for ci, cn in enumerate(CHUNKS):
                x_sb = x_tiles[ci]
                pt = ps.tile([CH, CN_MAX], F32, tag="pt", name="pt")[:, :cn]
                for ko in range(KO):
                    for s0 in range(0, cn, MM_N):
                        s1 = min(s0 + MM_N, cn)
                        # start/stop are per PSUM slab (bank region)
                        nc.tensor.matmul(
                            pt[:, s0:s1],
                            wh_sb[:, ko],
                            x_sb[:, ko, s0:s1],
                            start=(ko == 0),
                            stop=(ko == KO - 1),
                        )
                o_sb = op.tile([CH, CN_MAX], F16, tag="o", name="o_sb")[:, :cn]
                nc.scalar.activation(
                    o_sb[:],
                    pt[:],
                    mybir.ActivationFunctionType.Tanh,
                    bias=b_sb[:, 0:1],
                    scale=1.0,
                )
                # Store via the ACT engine's HWDGE ring: the store depends on
                # the activation anyway, and this keeps the Sync ring free to
                # stream x loads.
                nc.scalar.dma_start(out[:, n0 : n0 + cn], o_sb[:])
                n0 += cn
    nc.compile()
    return nc


def _get_nc():
    global _NC
    if _NC is None:
        _NC = _build_nc()
    return _NC


def _pack_x(shard8):
    # shard8 [BS, L] -> chunk-major [128, KO*BS]: per partition p, chunk c
    # occupies a contiguous (ko, n) block.
    shT = shard8.T  # [L, BS] view
    parts = []
    n0 = 0
    for cn in CHUNKS:
        blk = (
            shT[:, n0 : n0 + cn]
            .reshape(KO, 128, cn)
            .transpose(1, 0, 2)
            .reshape(128, KO * cn)
        )
        parts.append(blk)
        n0 += cn
    return np.ascontiguousarray(np.concatenate(parts, axis=1))


def _shard_inputs(features, W, b):
    features = np.ascontiguousarray(features, dtype=np.float32)
    W = np.ascontiguousarray(W, dtype=np.float32)
    b = np.ascontiguousarray(b, dtype=np.float32)

    # Wr[l, c] with c = k*A + a; fp16, device layout [p, ko, c]
    wr = W.transpose(2, 1, 0).reshape(L, CH)
    wr_h = wr.astype(np.float16)
    wh_dev = np.ascontiguousarray(wr_h.reshape(KO, 128, CH).transpose(1, 0, 2))
    b_dev = np.ascontiguousarray(b.transpose(1, 0).reshape(CH, 1))

    in_maps = []
    for i in range(NCORES):
        sh = features[i * BS : (i + 1) * BS]  # [BS, L]
        sh8 = sh.astype(NP_F8)
        in_maps.append(
            {
                "x8": _pack_x(sh8),
                "wh": wh_dev,
                "bias": b_dev,
            }
        )
    return in_maps


def _gather(results):
    out0 = np.empty((B, A), dtype=np.float32)
    out1 = np.empty((B, A), dtype=np.float32)
    for i, r in enumerate(results):
        arr = r["out"].T.astype(np.float32)  # [CH, BS] -> [BS, CH]
        out0[i * BS : (i + 1) * BS] = arr[:, :A]
        out1[i * BS : (i + 1) * BS] = arr[:, A:]
    return out0, out1


def _run(inputs, trace=False, trace_cores=None):
    nc = _get_nc()
    in_maps = _shard_inputs(inputs["features"], inputs["W"], inputs["b"])
    res = run_bass_kernel_spmd(
        nc,
        in_maps,
        core_ids=list(range(NCORES)),
        trace=trace,
        trace_cores=trace_cores,
    )
    return _gather(res.results), res


def kernel(features, W, b):
    (out0, out1), _ = _run({"features": features, "W": W, "b": b})
    return out0, out1
